# revision 47
# baseline (speedup 1.0000x reference)
"""Angular LSH bucketing kernel for 8 TRN2 NeuronCores.

Reference computation:
    scores  = mat @ proj_dir          # [b, h, n, 8]
    bits    = scores > 0
    bin_ids = sum(bits * 2^r)
    buckets = perm[bin_ids]           # perm is the Gray-code table

Sharding: data-parallel over batch*heads (64 -> 8 per core); projection
and tables replicated. Per core: 65536 rows of 64 dims.

Device strategy (v7, stream-bound at ~358 GB/s aggregate):
  - Host packs the bf16 image depth-major ([128, 32768]: partition p<64 =
    dim p of even rows, p>=64 = dims of odd rows, column q = row pair q),
    so every device DMA is a plain contiguous load -- no X-bar transpose
    (54 GB/s effective) and no per-tile weight loads (the v5/v6 designs
    bottomed out at ~256 x (LDWEIGHTS + isolated-matmul latency) ~ 45 us).
  - Inputs stream as 1MB transfers split across BOTH physical HWDGE rings
    (sync + scalar engines, ~358 GB/s aggregate); the 11KB const block
    rides the gpsimd SWDGE path so it never blocks a ring head.
  - Score matmuls keep the tiny [128, 32] weight block [pw | -pw]
    quasi-stationary and STREAM `a` as the moving operand (N=512 per MM),
    rotating PE column groups 0..3: a 4-tile supergroup (2048 pairs)
    fills psum [128, 512] with rows 32g+(0:16) = s, 32g+(16:32) = -s.
  - One ScalarE Sign pass (bias -TAU) per supergroup turns that psum into
    p = sign(s-TAU) / -q = -sign(s+TAU) bits (bf16).
  - A block-diagonal [128, 8] matmul collapses all 4 tiles' bits at once:
    word = bin + 256*gapcount - 127.5 lands on psum rows 32g+(0:8).
    gapcount counts scores inside (-TAU, TAU]; the +256 flag marks rows
    whose sign is not trustworthy at bf16 precision.
  - DVE adds 127.5, casts to int16; 8KB output pieces ship per supergroup
    (early ones on SWDGE, final four on the HWDGE rings).
  - Host maps words through perm and exactly recomputes flagged rows
    (~6% at TAU=0.08); measured end-to-end 0 mismatches.
"""

import numpy as np
import ml_dtypes

from concourse import bass, mybir
from concourse.bass_utils import run_bass_kernel_spmd

N_CORES = 8
B, H, N, D = 2, 32, 8192, 64
NPROJ = 8
ROWS_PER_CORE = (B * H // N_CORES) * N  # 65536
PAIRS = ROWS_PER_CORE // 2  # 32768
CHUNK_PAIRS = 4096
NCHUNK = PAIRS // CHUNK_PAIRS  # 8

F32 = mybir.dt.float32
BF16 = mybir.dt.bfloat16

_cache = {}



TAU = 0.08  # |score| threshold below which the host recomputes the row exactly
# (bf16 mat AND bf16 proj: score err std ~0.013, TAU ~ 6 sigma)


def _build_v7(pairs: int = PAIRS, chunk_pairs: int = CHUNK_PAIRS):
    """Streaming design: pw stationary-ish, `a` is the MOVING operand.

    v6's floor was ~256 x (LDWEIGHTS + isolated-MM latency) ~ 45 us: with
    `a` as the stationary operand every 32 pairs costs a weight load plus a
    ~178 ns matmul latency, and sub-array rotation caps concurrency at 4.
    Here each score matmul streams 512 pairs (N=512) through a tiny
    [128, 32] weight block [pw | -pw], rotating output col groups 0..3, so
    a 4-tile "supergroup" fills psum [128, 512] with rows 32g+(0:16) = s,
    32g+(16:32) = -s. One ACT Sign pass (bias -TAU) turns that into
    p = sign(s-TAU) (rows 0:16) and -q = -sign(s+TAU) (rows 16:32) as
    bf16 in SBUF. A second block-diagonal matmul wvec4 [128, 8]
    (alpha_r = (2^r-256)/2 on p-rows, -128 on -q-rows, columns 2i+j)
    collapses K=128 -> words for all 4 tiles at once: psum [8@32g, 512],
    word = bin + 256*gapcount - 127.5. DVE adds 127.5 and casts to i32;
    one full-width output DMA at the end. ~84 PE instructions total."""
    nchunk = pairs // chunk_pairs
    ngroup = pairs // 2048  # 4-tile supergroups of 2048 pairs
    assert ngroup == 16 and chunk_pairs == 4096
    nc = bass.Bass()
    a_d = nc.declare_dram_parameter("a", [128, pairs], BF16, isOutput=False)
    cst_d = nc.declare_dram_parameter("cst", [128, 52], BF16, isOutput=False)
    I16 = mybir.dt.int16
    out_d = nc.declare_dram_parameter("out", [4, 8, 2048], I16, isOutput=True)

    from contextlib import ExitStack

    with ExitStack() as ctx:
        ent = ctx.enter_context
        a_sb = ent(nc.sbuf_tensor("a_sb", [128, pairs], BF16))
        cst_sb = ent(nc.sbuf_tensor("cst_sb", [128, 52], BF16))
        pw_sb = cst_sb[:, 0:32]
        wv_sb = cst_sb[:, 32:40]
        tau_sb = cst_sb[:, 40:42].bitcast(F32)
        wv2_sb = cst_sb[:, 44:52]  # u/t-encoded weights for the final group
        bits = ent(nc.sbuf_tensor("bits", [128, 3, 512], BF16))  # triple buffer
        bi = ent(nc.sbuf_tensor("bi", [128, 2048], I16))
        # psum: score slots 0-3 at free [0:2048); words at free [2048:4096)
        ps = ent(nc.psum_tensor("ps", [128, 4096], F32))

        cs_sem = ent(nc.semaphore("cs_sem"))
        ch_sems = [ent(nc.semaphore(f"ch_sem{c}")) for c in range(nchunk)]
        ch15_sem = ent(nc.semaphore("ch15_sem"))
        mm_sem = ent(nc.semaphore("mm_sem"))
        act_sem = ent(nc.semaphore("act_sem"))
        wrd_sem = ent(nc.semaphore("wrd_sem"))
        b15_sem = ent(nc.semaphore("b15_sem"))
        dve_sem = ent(nc.semaphore("dve_sem"))
        out_sem = ent(nc.semaphore("out_sem"))

        def score_group(tensor, G):
            slot = G % 4
            for g in range(4):
                t = 4 * G + g
                mm = tensor.matmul(
                    ps[32 * g : 32 * (g + 1), 512 * slot : 512 * (slot + 1)],
                    pw_sb,
                    a_sb[:, 512 * t : 512 * (t + 1)],
                    start=True, stop=True, tile_position=(0, 32 * g),
                )
            mm.then_inc(mm_sem, 1)

        def word_mm(tensor, G):
            g, s = G % 4, G // 4
            tensor.matmul(
                ps[32 * g : 32 * g + 8, 2048 + 512 * s : 2048 + 512 * (s + 1)],
                wv2_sb if G == ngroup - 1 else wv_sb,
                bits[:, G % 3, :],
                start=True, stop=True, tile_position=(0, 32 * g),
            ).then_inc(wrd_sem, 1)

        with nc.Block() as block:

            # inputs as 1MB transfers (2 supergroups each) spread over
            # both HWDGE rings; the last 1MB is halved so G15's tail
            # ladder starts earlier. ch_sems[k] gates G=2k,2k+1
            # (ch15_sem gates G15 alone).
            def a_dma(eng, k):
                sl = slice(4096 * k, 4096 * (k + 1))
                eng.dma_start(out=a_sb[:, sl], in_=a_d[:, sl]).then_inc(
                    ch_sems[k], 16
                )

            # output piece (g, s) = words of supergroup G=4s+g, 8 KB,
            # ready after evac G; all but the last two ride the idle sync
            # ring and overlap the stream
            def out_piece(eng, g, s):
                eng.wait_ge(dve_sem, 4 * s + g + 1)
                eng.dma_start(
                    out=out_d[g][:, 512 * s : 512 * (s + 1)],
                    in_=bi[32 * g : 32 * g + 8, 512 * s : 512 * (s + 1)],
                ).then_inc(out_sem, 16)

            @block.gpsimd
            def _(gpsimd):
                # SWDGE path: const load and the early output pieces,
                # all off the HWDGE input rings
                gpsimd.dma_start(out=cst_sb[:], in_=cst_d[:]).then_inc(cs_sem, 16)
                for s in range(3):
                    for g in range(4):
                        out_piece(gpsimd, g, s)

            @block.sync
            def _(sync):
                for k in (0, 2, 4, 6):
                    a_dma(sync, k)
                out_piece(sync, 0, 3)
                out_piece(sync, 2, 3)
                sync.wait_ge(out_sem, 256)

            @block.tensor
            def _(tensor):
                tensor.wait_ge(cs_sem, 16)
                for G in range(ngroup):
                    if G == 15:
                        tensor.wait_ge(ch15_sem, 16)
                    elif G % 2 == 0:
                        tensor.wait_ge(ch_sems[G // 2], 16)
                    if G >= 4:
                        tensor.wait_ge(act_sem, G - 3)  # psum slot G%4 free
                    score_group(tensor, G)
                    if G >= 1:
                        tensor.wait_ge(act_sem, G)  # bits(G-1) ready
                        word_mm(tensor, G - 1)
                tensor.wait_ge(b15_sem, 1)
                word_mm(tensor, ngroup - 1)

            @block.scalar
            def _(scalar):
                for k in (1, 3, 5):
                    a_dma(scalar, k)
                sl = slice(28672, 30720)  # G14 half
                scalar.dma_start(out=a_sb[:, sl], in_=a_d[:, sl]).then_inc(
                    ch_sems[7], 16
                )
                sl = slice(30720, 32768)  # G15 half
                scalar.dma_start(out=a_sb[:, sl], in_=a_d[:, sl]).then_inc(
                    ch15_sem, 16
                )
                for G in range(ngroup - 1):
                    scalar.wait_ge(mm_sem, G + 1)
                    if G >= 3:
                        scalar.wait_ge(wrd_sem, G - 2)  # bits buf G%3 free
                    slot = G % 4
                    scalar.activation(
                        bits[:, G % 3, :],
                        ps[:, 512 * slot : 512 * (slot + 1)],
                        mybir.ActivationFunctionType.Sign,
                        bias=tau_sb,
                    ).then_inc(act_sem, 1)
                out_piece(scalar, 1, 3)
                out_piece(scalar, 3, 3)
                # engine stream ends here; sync holds the final out gate

            @block.vector
            def _(vector):
                for G in range(ngroup - 2):
                    vector.wait_ge(wrd_sem, G + 1)
                    g, s = G % 4, G // 4
                    vector.tensor_scalar_add(
                        bi[32 * g : 32 * g + 8, 512 * s : 512 * (s + 1)],
                        ps[32 * g : 32 * g + 8, 2048 + 512 * s : 2048 + 512 * (s + 1)],
                        127.5,
                    ).then_inc(dve_sem, 1)
                # final group: bits on DVE (skips the ACT FIFO in the tail)
                vector.wait_ge(mm_sem, ngroup)
                vector.tensor_single_scalar(
                    bits[:, (ngroup - 1) % 3, :],
                    ps[:, 512 * ((ngroup - 1) % 4) : 512 * ((ngroup - 1) % 4 + 1)],
                    TAU, mybir.AluOpType.is_gt,
                ).then_inc(b15_sem, 1)
                for G in (ngroup - 2, ngroup - 1):
                    vector.wait_ge(wrd_sem, G + 1)
                    g, s = G % 4, G // 4
                    vector.tensor_scalar_add(
                        bi[32 * g : 32 * g + 8, 512 * s : 512 * (s + 1)],
                        ps[32 * g : 32 * g + 8, 2048 + 512 * s : 2048 + 512 * (s + 1)],
                        2048.0 if G == ngroup - 1 else 127.5,
                    ).then_inc(dve_sem, 1)
    return nc


def _prep_v7(mat, proj_dir):
    bf16 = ml_dtypes.bfloat16
    flat = np.ascontiguousarray(mat.reshape(B * H, N, D), dtype=np.float32)
    a_full = flat.astype(bf16)

    p = np.asarray(proj_dir, dtype=np.float32).reshape(D, NPROJ)
    pa = p.astype(bf16)
    pw = np.zeros((128, 32), dtype=bf16)
    pw[0:64, 0:8] = pa
    pw[64:128, 8:16] = pa
    pw[:, 16:32] = -pw[:, 0:16]

    alpha = (2.0 ** np.arange(NPROJ, dtype=np.float32) - 256.0) / 2.0
    wv = np.zeros((128, 8), dtype=np.float32)
    for i in range(4):
        for j in range(2):
            for r in range(NPROJ):
                wv[32 * i + 8 * j + r, 2 * i + j] = alpha[r]
                wv[32 * i + 16 + 8 * j + r, 2 * i + j] = -128.0
    wv = wv.astype(bf16)

    wv2 = np.zeros((128, 8), dtype=np.float32)
    for i in range(4):
        for j in range(2):
            for r in range(NPROJ):
                wv2[32 * i + 8 * j + r, 2 * i + j] = 2.0 ** r - 256.0
                wv2[32 * i + 16 + 8 * j + r, 2 * i + j] = -256.0

    cst = np.zeros((128, 52), dtype=bf16)
    cst[:, 0:32] = pw
    cst[:, 32:40] = wv
    cst[:, 40:42] = np.full((128, 1), -TAU, dtype=np.float32).view(bf16)
    cst[:, 44:52] = wv2.astype(bf16)

    bh_per_core = B * H // N_CORES
    in_maps = []
    for i in range(N_CORES):
        sh = a_full[i * bh_per_core : (i + 1) * bh_per_core]
        a = sh.reshape(PAIRS, 128)
        aT = np.ascontiguousarray(a.T)  # [128, PAIRS]
        in_maps.append({"a": aT, "cst": cst})
    return in_maps


def _decode_v7(dev_out):
    """[4, 8, 2048] device words -> [65536] per-core row-ordered words.

    Word of tile 16s+4g+i, pair tile*512+n, parity j sits at
    dev[g, 2i+j, 512s + n]."""
    v = dev_out.reshape(4, 4, 2, 4, 512)               # (g, i, j, s, n)
    return np.ascontiguousarray(v.transpose(3, 0, 1, 4, 2)).reshape(-1)


def kernel(mat, proj_dir, perm, enc_vec, _trace=False, _tmpdir=None):
    enc = np.asarray(enc_vec).reshape(-1).astype(np.int64)
    perm_arr = np.asarray(perm).reshape(-1).astype(np.int64)
    std_enc = enc.shape[0] == NPROJ and np.array_equal(enc, 2 ** np.arange(NPROJ))
    if not (std_enc and perm_arr.shape[0] == 256):
        # Pathological setup the device word-packing doesn't cover (the
        # harness never hits this): plain host computation.
        flat = np.ascontiguousarray(mat.reshape(B * H * N, D), dtype=np.float64)
        p = np.asarray(proj_dir, dtype=np.float64).reshape(D, NPROJ)
        bits = (flat @ p > 0).astype(np.int64)
        bins = (bits * enc).sum(-1)
        out = perm_arr[bins].reshape(B, H, N).astype(np.int32)
        return (out, None) if _trace else out

    if "v7" not in _cache:
        _cache["v7"] = _build_v7()
    nc = _cache["v7"]

    in_maps = _prep_v7(mat, proj_dir)
    res = run_bass_kernel_spmd(
        nc, in_maps, core_ids=list(range(N_CORES)), trace=_trace, tmpdir=_tmpdir
    )
    word = np.concatenate(
        [_decode_v7(np.asarray(r["out"])) for r in res.results]
    ).astype(np.int64)
    buckets = perm_arr[word & 255]  # device emits raw bin ids
    flagged = word >= 256           # device min|score| < TAU

    # Host fix-up: rows whose smallest |bf16 score| is inside the rounding
    # envelope get recomputed exactly.
    idx = np.nonzero(flagged)[0]
    if idx.size:
        flat = np.ascontiguousarray(mat.reshape(B * H * N, D), dtype=np.float32)
        p = np.asarray(proj_dir, dtype=np.float32).reshape(D, NPROJ)
        sc = flat[idx] @ p
        bits = (sc > 0).astype(np.int64)
        bins = (bits * enc).sum(-1)
        buckets[idx] = perm_arr[bins]
    out = buckets.reshape(B, H, N).astype(np.int32)
    if _trace:
        return out, res
    return out



# revision 48
# speedup vs baseline: 1.0196x; 1.0196x over previous
"""Angular LSH bucketing kernel for 8 TRN2 NeuronCores.

Reference computation:
    scores  = mat @ proj_dir          # [b, h, n, 8]
    bits    = scores > 0
    bin_ids = sum(bits * 2^r)
    buckets = perm[bin_ids]           # perm is the Gray-code table

Sharding: data-parallel over batch*heads (64 -> 8 per core); projection
and tables replicated. Per core: 65536 rows of 64 dims.

Device strategy (v7, stream-bound at ~358 GB/s aggregate):
  - Host packs the bf16 image depth-major ([128, 32768]: partition p<64 =
    dim p of even rows, p>=64 = dims of odd rows, column q = row pair q),
    so every device DMA is a plain contiguous load -- no X-bar transpose
    (54 GB/s effective) and no per-tile weight loads (the v5/v6 designs
    bottomed out at ~256 x (LDWEIGHTS + isolated-matmul latency) ~ 45 us).
  - Inputs stream as 1MB transfers split across BOTH physical HWDGE rings
    (sync + scalar engines, ~358 GB/s aggregate); the 11KB const block
    rides the gpsimd SWDGE path so it never blocks a ring head.
  - Score matmuls keep the tiny [128, 32] weight block [pw | -pw]
    quasi-stationary and STREAM `a` as the moving operand (N=512 per MM),
    rotating PE column groups 0..3: a 4-tile supergroup (2048 pairs)
    fills psum [128, 512] with rows 32g+(0:16) = s, 32g+(16:32) = -s.
  - One ScalarE Sign pass (bias -TAU) per supergroup turns that psum into
    p = sign(s-TAU) / -q = -sign(s+TAU) bits (bf16).
  - A block-diagonal [128, 8] matmul collapses all 4 tiles' bits at once:
    word = bin + 256*gapcount - 127.5 lands on psum rows 32g+(0:8).
    gapcount counts scores inside (-TAU, TAU]; the +256 flag marks rows
    whose sign is not trustworthy at bf16 precision.
  - DVE adds 127.5, casts to int16; 8KB output pieces ship per supergroup
    (early ones on SWDGE, final four on the HWDGE rings).
  - Host maps words through perm and exactly recomputes flagged rows
    (~6% at TAU=0.08); measured end-to-end 0 mismatches.
"""

import numpy as np
import ml_dtypes

from concourse import bass, mybir
from concourse.bass_utils import run_bass_kernel_spmd

N_CORES = 8
B, H, N, D = 2, 32, 8192, 64
NPROJ = 8
ROWS_PER_CORE = (B * H // N_CORES) * N  # 65536
PAIRS = ROWS_PER_CORE // 2  # 32768
CHUNK_PAIRS = 4096
NCHUNK = PAIRS // CHUNK_PAIRS  # 8

F32 = mybir.dt.float32
BF16 = mybir.dt.bfloat16

_cache = {}



TAU = 0.08  # |score| threshold below which the host recomputes the row exactly
# (bf16 mat AND bf16 proj: score err std ~0.013, TAU ~ 6 sigma)


def _build_v7(pairs: int = PAIRS, chunk_pairs: int = CHUNK_PAIRS):
    """Streaming design: pw stationary-ish, `a` is the MOVING operand.

    v6's floor was ~256 x (LDWEIGHTS + isolated-MM latency) ~ 45 us: with
    `a` as the stationary operand every 32 pairs costs a weight load plus a
    ~178 ns matmul latency, and sub-array rotation caps concurrency at 4.
    Here each score matmul streams 512 pairs (N=512) through a tiny
    [128, 32] weight block [pw | -pw], rotating output col groups 0..3, so
    a 4-tile "supergroup" fills psum [128, 512] with rows 32g+(0:16) = s,
    32g+(16:32) = -s. One ACT Sign pass (bias -TAU) turns that into
    p = sign(s-TAU) (rows 0:16) and -q = -sign(s+TAU) (rows 16:32) as
    bf16 in SBUF. A second block-diagonal matmul wvec4 [128, 8]
    (alpha_r = (2^r-256)/2 on p-rows, -128 on -q-rows, columns 2i+j)
    collapses K=128 -> words for all 4 tiles at once: psum [8@32g, 512],
    word = bin + 256*gapcount - 127.5. DVE adds 127.5 and casts to i32;
    one full-width output DMA at the end. ~84 PE instructions total."""
    nchunk = pairs // chunk_pairs
    ngroup = pairs // 2048  # 4-tile supergroups of 2048 pairs
    assert ngroup == 16 and chunk_pairs == 4096
    nc = bass.Bass()
    a_d = nc.declare_dram_parameter("a", [128, pairs], BF16, isOutput=False)
    cst_d = nc.declare_dram_parameter("cst", [128, 52], BF16, isOutput=False)
    I16 = mybir.dt.int16
    out_d = nc.declare_dram_parameter("out", [4, 8, 2048], I16, isOutput=True)

    from contextlib import ExitStack

    with ExitStack() as ctx:
        ent = ctx.enter_context
        a_sb = ent(nc.sbuf_tensor("a_sb", [128, pairs], BF16))
        cst_sb = ent(nc.sbuf_tensor("cst_sb", [128, 52], BF16))
        pw_sb = cst_sb[:, 0:32]
        wv_sb = cst_sb[:, 32:40]
        tau_sb = cst_sb[:, 40:42].bitcast(F32)
        wv2_sb = cst_sb[:, 44:52]  # u/t-encoded weights for the final group
        bits = ent(nc.sbuf_tensor("bits", [128, 3, 512], BF16))  # triple buffer
        bi = ent(nc.sbuf_tensor("bi", [128, 2048], I16))
        # psum: score slots 0-3 at free [0:2048); words at free [2048:4096)
        ps = ent(nc.psum_tensor("ps", [128, 4096], F32))

        cs_sem = ent(nc.semaphore("cs_sem"))
        ch_sems = [ent(nc.semaphore(f"ch_sem{c}")) for c in range(nchunk)]
        ch15_sem = ent(nc.semaphore("ch15_sem"))
        mm_sem = ent(nc.semaphore("mm_sem"))
        act_sem = ent(nc.semaphore("act_sem"))
        wrd_sem = ent(nc.semaphore("wrd_sem"))
        b15_sem = ent(nc.semaphore("b15_sem"))
        dve_sem = ent(nc.semaphore("dve_sem"))
        out_sem = ent(nc.semaphore("out_sem"))

        def score_group(tensor, G):
            slot = G % 4
            for g in range(4):
                t = 4 * G + g
                mm = tensor.matmul(
                    ps[32 * g : 32 * (g + 1), 512 * slot : 512 * (slot + 1)],
                    pw_sb,
                    a_sb[:, 512 * t : 512 * (t + 1)],
                    start=True, stop=True, tile_position=(0, 32 * g),
                )
            mm.then_inc(mm_sem, 1)

        def word_mm(tensor, G):
            g, s = G % 4, G // 4
            tensor.matmul(
                ps[32 * g : 32 * g + 8, 2048 + 512 * s : 2048 + 512 * (s + 1)],
                wv2_sb if G == ngroup - 1 else wv_sb,
                bits[:, G % 3, :],
                start=True, stop=True, tile_position=(0, 32 * g),
            ).then_inc(wrd_sem, 1)

        with nc.Block() as block:

            # inputs as 1MB transfers (2 supergroups each) spread over
            # both HWDGE rings; the last 1MB is halved so G15's tail
            # ladder starts earlier. ch_sems[k] gates G=2k,2k+1
            # (ch15_sem gates G15 alone).
            def a_dma(eng, k):
                sl = slice(4096 * k, 4096 * (k + 1))
                eng.dma_start(out=a_sb[:, sl], in_=a_d[:, sl]).then_inc(
                    ch_sems[k], 16
                )

            # output piece (g, s) = words of supergroup G=4s+g, 8 KB,
            # ready after evac G; all but the last two ride the idle sync
            # ring and overlap the stream
            def out_piece(eng, g, s):
                eng.wait_ge(dve_sem, 4 * s + g + 1)
                eng.dma_start(
                    out=out_d[g][:, 512 * s : 512 * (s + 1)],
                    in_=bi[32 * g : 32 * g + 8, 512 * s : 512 * (s + 1)],
                ).then_inc(out_sem, 16)

            @block.gpsimd
            def _(gpsimd):
                # SWDGE path: const load and the early output pieces,
                # all off the HWDGE input rings
                gpsimd.dma_start(out=cst_sb[:], in_=cst_d[:]).then_inc(cs_sem, 16)
                for s in range(3):
                    for g in range(4):
                        out_piece(gpsimd, g, s)

            @block.sync
            def _(sync):
                for k in (0, 2, 4, 6):
                    a_dma(sync, k)
                sl = slice(28672, 30720)  # G14 half
                sync.dma_start(out=a_sb[:, sl], in_=a_d[:, sl]).then_inc(
                    ch_sems[7], 16
                )
                out_piece(sync, 0, 3)
                out_piece(sync, 2, 3)
                sync.wait_ge(out_sem, 256)

            @block.tensor
            def _(tensor):
                tensor.wait_ge(cs_sem, 16)
                for G in range(ngroup):
                    if G == 15:
                        tensor.wait_ge(ch15_sem, 16)
                    elif G % 2 == 0:
                        tensor.wait_ge(ch_sems[G // 2], 16)
                    if G >= 4:
                        tensor.wait_ge(act_sem, G - 3)  # psum slot G%4 free
                    score_group(tensor, G)
                    if G >= 1:
                        tensor.wait_ge(act_sem, G)  # bits(G-1) ready
                        word_mm(tensor, G - 1)
                tensor.wait_ge(b15_sem, 1)
                word_mm(tensor, ngroup - 1)

            @block.scalar
            def _(scalar):
                for k in (1, 3, 5):
                    a_dma(scalar, k)
                sl = slice(30720, 32768)  # G15 half
                scalar.dma_start(out=a_sb[:, sl], in_=a_d[:, sl]).then_inc(
                    ch15_sem, 16
                )
                for G in range(ngroup - 1):
                    scalar.wait_ge(mm_sem, G + 1)
                    if G >= 3:
                        scalar.wait_ge(wrd_sem, G - 2)  # bits buf G%3 free
                    slot = G % 4
                    scalar.activation(
                        bits[:, G % 3, :],
                        ps[:, 512 * slot : 512 * (slot + 1)],
                        mybir.ActivationFunctionType.Sign,
                        bias=tau_sb,
                    ).then_inc(act_sem, 1)
                out_piece(scalar, 1, 3)
                out_piece(scalar, 3, 3)
                # engine stream ends here; sync holds the final out gate

            @block.vector
            def _(vector):
                for G in range(ngroup - 2):
                    vector.wait_ge(wrd_sem, G + 1)
                    g, s = G % 4, G // 4
                    vector.tensor_scalar_add(
                        bi[32 * g : 32 * g + 8, 512 * s : 512 * (s + 1)],
                        ps[32 * g : 32 * g + 8, 2048 + 512 * s : 2048 + 512 * (s + 1)],
                        127.5,
                    ).then_inc(dve_sem, 1)
                # final group: bits on DVE (skips the ACT FIFO in the tail)
                vector.wait_ge(mm_sem, ngroup)
                vector.tensor_single_scalar(
                    bits[:, (ngroup - 1) % 3, :],
                    ps[:, 512 * ((ngroup - 1) % 4) : 512 * ((ngroup - 1) % 4 + 1)],
                    TAU, mybir.AluOpType.is_gt,
                ).then_inc(b15_sem, 1)
                for G in (ngroup - 2, ngroup - 1):
                    vector.wait_ge(wrd_sem, G + 1)
                    g, s = G % 4, G // 4
                    vector.tensor_scalar_add(
                        bi[32 * g : 32 * g + 8, 512 * s : 512 * (s + 1)],
                        ps[32 * g : 32 * g + 8, 2048 + 512 * s : 2048 + 512 * (s + 1)],
                        2048.0 if G == ngroup - 1 else 127.5,
                    ).then_inc(dve_sem, 1)
    return nc


def _prep_v7(mat, proj_dir):
    bf16 = ml_dtypes.bfloat16
    flat = np.ascontiguousarray(mat.reshape(B * H, N, D), dtype=np.float32)
    a_full = flat.astype(bf16)

    p = np.asarray(proj_dir, dtype=np.float32).reshape(D, NPROJ)
    pa = p.astype(bf16)
    pw = np.zeros((128, 32), dtype=bf16)
    pw[0:64, 0:8] = pa
    pw[64:128, 8:16] = pa
    pw[:, 16:32] = -pw[:, 0:16]

    alpha = (2.0 ** np.arange(NPROJ, dtype=np.float32) - 256.0) / 2.0
    wv = np.zeros((128, 8), dtype=np.float32)
    for i in range(4):
        for j in range(2):
            for r in range(NPROJ):
                wv[32 * i + 8 * j + r, 2 * i + j] = alpha[r]
                wv[32 * i + 16 + 8 * j + r, 2 * i + j] = -128.0
    wv = wv.astype(bf16)

    wv2 = np.zeros((128, 8), dtype=np.float32)
    for i in range(4):
        for j in range(2):
            for r in range(NPROJ):
                wv2[32 * i + 8 * j + r, 2 * i + j] = 2.0 ** r - 256.0
                wv2[32 * i + 16 + 8 * j + r, 2 * i + j] = -256.0

    cst = np.zeros((128, 52), dtype=bf16)
    cst[:, 0:32] = pw
    cst[:, 32:40] = wv
    cst[:, 40:42] = np.full((128, 1), -TAU, dtype=np.float32).view(bf16)
    cst[:, 44:52] = wv2.astype(bf16)

    bh_per_core = B * H // N_CORES
    in_maps = []
    for i in range(N_CORES):
        sh = a_full[i * bh_per_core : (i + 1) * bh_per_core]
        a = sh.reshape(PAIRS, 128)
        aT = np.ascontiguousarray(a.T)  # [128, PAIRS]
        in_maps.append({"a": aT, "cst": cst})
    return in_maps


def _decode_v7(dev_out):
    """[4, 8, 2048] device words -> [65536] per-core row-ordered words.

    Word of tile 16s+4g+i, pair tile*512+n, parity j sits at
    dev[g, 2i+j, 512s + n]."""
    v = dev_out.reshape(4, 4, 2, 4, 512)               # (g, i, j, s, n)
    return np.ascontiguousarray(v.transpose(3, 0, 1, 4, 2)).reshape(-1)


def kernel(mat, proj_dir, perm, enc_vec, _trace=False, _tmpdir=None):
    enc = np.asarray(enc_vec).reshape(-1).astype(np.int64)
    perm_arr = np.asarray(perm).reshape(-1).astype(np.int64)
    std_enc = enc.shape[0] == NPROJ and np.array_equal(enc, 2 ** np.arange(NPROJ))
    if not (std_enc and perm_arr.shape[0] == 256):
        # Pathological setup the device word-packing doesn't cover (the
        # harness never hits this): plain host computation.
        flat = np.ascontiguousarray(mat.reshape(B * H * N, D), dtype=np.float64)
        p = np.asarray(proj_dir, dtype=np.float64).reshape(D, NPROJ)
        bits = (flat @ p > 0).astype(np.int64)
        bins = (bits * enc).sum(-1)
        out = perm_arr[bins].reshape(B, H, N).astype(np.int32)
        return (out, None) if _trace else out

    if "v7" not in _cache:
        _cache["v7"] = _build_v7()
    nc = _cache["v7"]

    in_maps = _prep_v7(mat, proj_dir)
    res = run_bass_kernel_spmd(
        nc, in_maps, core_ids=list(range(N_CORES)), trace=_trace, tmpdir=_tmpdir
    )
    word = np.concatenate(
        [_decode_v7(np.asarray(r["out"])) for r in res.results]
    ).astype(np.int64)
    buckets = perm_arr[word & 255]  # device emits raw bin ids
    flagged = word >= 256           # device min|score| < TAU

    # Host fix-up: rows whose smallest |bf16 score| is inside the rounding
    # envelope get recomputed exactly.
    idx = np.nonzero(flagged)[0]
    if idx.size:
        flat = np.ascontiguousarray(mat.reshape(B * H * N, D), dtype=np.float32)
        p = np.asarray(proj_dir, dtype=np.float32).reshape(D, NPROJ)
        sc = flat[idx] @ p
        bits = (sc > 0).astype(np.int64)
        bins = (bits * enc).sum(-1)
        buckets[idx] = perm_arr[bins]
    out = buckets.reshape(B, H, N).astype(np.int32)
    if _trace:
        return out, res
    return out

